# revision 54
# baseline (speedup 1.0000x reference)
"""CFConv (SchNet-style continuous-filter conv with per-target segment softmax)
on 8 Trainium2 NeuronCores.

Sharding: target-bucketed edge parallelism. Core k owns target nodes
[k*6250, (k+1)*6250). The host buckets+sorts edges by target and bin-packs
targets into 200 groups of <=40 target slots x 512 edge slots (4 tiles of
128), an identical static structure on every core (SPMD-safe, no
collectives: each core owns disjoint targets).

Device pipeline per 2-group window:
  - edge MLP layer 1 on PE (attr_T @ W1), tanh+b1 on ACT
  - layer 2 per 128-edge tile on PE: [ew | ew*attn] = h_tile^T @ [W2 | W2*attn],
    accumulated on top of a PSUM preload of [b2 | b2*attn] (ones outer product)
  - messages msg = xg * (ew + b2) and scores = sum_f xg*(ew*attn + b2*attn)
    as batched DVE ops (xg = host-pregathered x[source])
  - softmax without max-subtraction (scores are O(10); exp safe in f32)
  - segment sums via one-hot matmuls: numer[t,:] | denom[t] accumulate in
    PSUM (one-hot lhsT scaled by exp(score) so pads contribute nothing)
  - attention weights = ex * (O @ (1/denom)) via batched TT+reduce
"""

import math
import sys

import numpy as np

sys.path.insert(0, "/opt/trn_rl_repo")

import ml_dtypes  # noqa: E402

BF16 = ml_dtypes.bfloat16

# problem constants (hardcoded per spec)
N_NODES = 50000
N_EDGES = 800000
D_IN = 64
D_EDGE = 32
D_HID = 128
N_CORES = 8
T_PER_CORE = N_NODES // N_CORES  # 6250

# device layout constants
G_TGT = 40          # target slots per group
G_TILES = 4         # 128-edge tiles per group
E_GROUP = G_TILES * 128  # 512 edge slots per group
N_GROUPS = 200      # groups per core; 200*40 = 8000 target slots
N_TILES = N_GROUPS * G_TILES  # 800
E_CAP = N_GROUPS * E_GROUP    # 102400
COLG = 2            # psum column groups (partition offsets 0 / 64)
DEPTH = 3           # psum drain depth
GPD = COLG * DEPTH  # groups per drain window = 6
N_WIN = math.ceil(N_GROUPS / GPD)  # 34 (last window partial: 2 groups)
NCOL = N_GROUPS // COLG  # 100 columns in drained arena (= g//2)
PSW = 65            # psum width per group: 64 numer + 1 denom

_CACHED = {}


def _pack_core(counts):
    """Greedy edge-limited bin packing of sorted targets into groups."""
    g = np.zeros(T_PER_CORE, np.int32)
    s = np.zeros(T_PER_CORE, np.int32)
    cur_g, cur_s, cur_e = 0, 0, 0
    for t in range(T_PER_CORE):
        c = counts[t]
        if cur_s >= G_TGT or cur_e + c > E_GROUP:
            cur_g += 1
            cur_s, cur_e = 0, 0
        assert cur_g < N_GROUPS, "bin packing overflow; increase N_GROUPS"
        assert c <= E_GROUP, "single target exceeds group capacity"
        g[t] = cur_g
        s[t] = cur_s
        cur_s += 1
        cur_e += c
    return g, s


def _prep_core(k, x, src, tgt, edge_attr):
    """Build one core's input arrays + reassembly metadata."""
    sel = np.nonzero((tgt >= k * T_PER_CORE) & (tgt < (k + 1) * T_PER_CORE))[0]
    tl = tgt[sel] - k * T_PER_CORE
    order = np.argsort(tl, kind="stable")
    eids = sel[order].astype(np.int64)
    tl = tl[order]
    counts = np.bincount(tl, minlength=T_PER_CORE)
    tg, ts = _pack_core(counts)

    eg = tg[tl]                       # per-edge group
    es = ts[tl]                       # per-edge slot in group
    grp_start = np.zeros(N_GROUPS + 1, np.int64)
    np.add.at(grp_start, eg + 1, 1)
    grp_start = np.cumsum(grp_start)
    pos_in_grp = np.arange(len(eids)) - grp_start[eg]

    j = eg * E_GROUP + pos_in_grp
    p = (pos_in_grp % 128).astype(np.int64)          # partition
    T = eg * G_TILES + pos_in_grp // 128             # global tile index

    attr_t = np.zeros((D_EDGE, E_CAP), BF16)
    attr_t[:, j] = edge_attr[eids].T

    xs = x[src[eids]]                                 # [e, 64] host gather
    # [128, N_TILES, 65]: cols 0:64 = x[src], col 64 = 1.0 (denom column)
    xgext = np.zeros((128, N_TILES, 65), BF16)
    xgext[p, T, :64] = xs
    xgext[p, T, 64] = BF16(1.0)

    O = np.zeros((128, N_TILES, G_TGT), BF16)
    O[p, T, es] = BF16(1.0)

    # reassembly: target t -> (partition, col) in outv [128, NCOL, 64]
    t_part = 64 * (tg % COLG) + ts
    t_col = tg // COLG
    return {
        "attr_t": attr_t,
        "xgext": xgext,
        "onehot": O,
    }, {
        "eids": eids,
        "e_p": p,
        "e_T": T,
        "t_part": t_part,
        "t_col": t_col,
    }


def _build_graph():
    """Build the SPMD bass graph (shared across cores)."""
    import contextlib

    from concourse import bacc, bass, mybir, tile

    f32 = mybir.dt.float32
    bf16 = mybir.dt.bfloat16
    Alu = mybir.AluOpType
    Act = mybir.ActivationFunctionType

    nc = bacc.Bacc(None, target_bir_lowering=False)

    attr_d = nc.declare_dram_parameter("attr_t", [D_EDGE, E_CAP], bf16, isOutput=False)
    xg_d = nc.declare_dram_parameter("xgext", [128, N_TILES, 65], bf16, isOutput=False)
    oh_d = nc.declare_dram_parameter("onehot", [128, N_TILES, G_TGT], bf16, isOutput=False)
    w1_d = nc.declare_dram_parameter("W1", [D_EDGE, D_HID], bf16, isOutput=False)
    b1_d = nc.declare_dram_parameter("b1", [D_HID, 1], f32, isOutput=False)
    w2c_d = nc.declare_dram_parameter("W2cat", [D_HID, 64], bf16, isOutput=False)
    # [1, 512]: b2 repeated for the 8 tiles of one window (psum preload row)
    b2r_d = nc.declare_dram_parameter("b2row", [1, 512], bf16, isOutput=False)
    attn_d = nc.declare_dram_parameter("attnv", [1, D_IN], bf16, isOutput=False)

    outv_d = nc.declare_dram_parameter("outv", [128, NCOL, D_IN], f32, isOutput=True)
    attnw_d = nc.declare_dram_parameter("attnw", [128, N_TILES], f32, isOutput=True)

    rdflat_d = nc.dram_tensor("rdscratch", [1, N_GROUPS * G_TGT], bf16)

    W2G = 2  # groups per pipeline window (psum tiles sized for 2 groups)
    n_w2 = N_GROUPS // W2G  # 100

    with tile.TileContext(nc) as tc:
        with contextlib.ExitStack() as ctx:
            singles = ctx.enter_context(tc.tile_pool(name="singles", bufs=1))
            loads = ctx.enter_context(tc.tile_pool(name="loads", bufs=8))
            work = ctx.enter_context(tc.tile_pool(name="work", bufs=2))
            p2p = ctx.enter_context(tc.tile_pool(name="p2p", bufs=2))
            opp = ctx.enter_context(tc.tile_pool(name="opp", bufs=3))
            psum_h = ctx.enter_context(tc.tile_pool(name="psum_h", bufs=2, space="PSUM"))
            psum_e = ctx.enter_context(tc.tile_pool(name="psum_e", bufs=2, space="PSUM"))
            psum_n = ctx.enter_context(tc.tile_pool(name="psum_n", bufs=2, space="PSUM"))

            # persistent tensors
            w1_s = singles.tile([D_EDGE, D_HID], bf16)
            b1_s = singles.tile([D_HID, 1], f32)
            w2c_s = singles.tile([D_HID, 64], bf16)
            b2r_s = singles.tile([1, 512], bf16)
            ones_s = singles.tile([1, 128], bf16)
            attnrep = singles.tile([128, D_IN], bf16)
            _attnrep = attnrep[:]
            oh_s = singles.tile([128, N_TILES, G_TGT], bf16)     # 6.6 MB
            ex_s = singles.tile([128, N_TILES], f32)
            aw_raw = singles.tile([128, N_TILES], f32)
            arena = singles.tile([128, N_WIN, DEPTH, PSW], f32)  # 3.4 MB
            rd_s = singles.tile([128, N_WIN * DEPTH], bf16)
            outv_s = singles.tile([128, NCOL, D_IN], f32)        # 3.3 MB

            nc.sync.dma_start(out=w1_s[:], in_=w1_d[:])
            nc.sync.dma_start(out=b1_s[:], in_=b1_d[:])
            nc.sync.dma_start(out=w2c_s[:], in_=w2c_d[:])
            nc.sync.dma_start(out=b2r_s[:], in_=b2r_d[:])
            nc.vector.memset(ones_s[:], 1.0)
            nc.sync.dma_start(
                out=attnrep[:],
                in_=bass.AP(tensor=attn_d, offset=0, ap=[[0, 128], [1, D_IN]]),
            )


            # ---- main pass: per 2-group pipeline window ----
            msg_tiles = [None] * N_GROUPS

            def fetch_oh(d):
                g0 = d * GPD
                gn = min(GPD, N_GROUPS - g0)
                ntile = gn * G_TILES
                nc.sync.dma_start(
                    out=oh_s[:, g0 * G_TILES:g0 * G_TILES + ntile, :],
                    in_=oh_d[:, g0 * G_TILES:g0 * G_TILES + ntile, :],
                )

            fetch_oh(0)

            def issue_drain(d):
                g0 = d * GPD
                gn = min(GPD, N_GROUPS - g0)
                ntile = gn * G_TILES
                if d + 1 < N_WIN:
                    fetch_oh(d + 1)
                op_t = op_arenas[d]
                num_ps = psum_n.tile([128, DEPTH, PSW], f32, space="PSUM")
                # interleave the two column groups so adjacent matmuls sit in
                # different array quadrants and pipeline (~4ns apart)
                for gi2 in range((gn + 1) // 2):
                    for t in range(G_TILES):
                        for j in range(2):
                            gi = 2 * gi2 + j
                            if gi >= gn:
                                continue
                            g = g0 + gi
                            jcg = g % COLG
                            dcol = (g // COLG) % DEPTH
                            ps = num_ps[64 * jcg:64 * jcg + G_TGT, dcol, :]
                            mt = msg_tiles[(g // W2G) * W2G]
                            mi = g % W2G
                            nc.tensor.matmul(
                                out=ps[0:G_TGT, :],
                                lhsT=op_t[:, gi * G_TILES + t, :],
                                rhs=mt[:, mi * G_TILES + t, :],
                                start=(t == 0),
                                stop=(t == G_TILES - 1),
                                tile_position=(0, 64 * jcg),
                                skip_group_check=True,
                            )
                nc.scalar.copy(out=arena[:, d, :, :], in_=num_ps[:])

                # --- incremental post: rd, out divide, pass-2 attn weights ---
                ncd = (gn + COLG - 1) // COLG  # valid out columns this window
                nc.gpsimd.tensor_scalar_max(
                    out=rd_s[:, DEPTH * d:DEPTH * d + DEPTH],
                    in0=arena[:, d, :, 64],
                    scalar1=1e-30,
                )
                with nc.allow_low_precision(reason="bf16 softmax denominators, 2e-2 tolerance"):
                    nc.vector.reciprocal(
                        out=rd_s[:, DEPTH * d:DEPTH * d + DEPTH],
                        in_=rd_s[:, DEPTH * d:DEPTH * d + DEPTH],
                    )
                _rdap = rd_s[:]
                rdb = bass.AP(
                    tensor=_rdap.tensor,
                    offset=_rdap.offset + DEPTH * d,
                    ap=[[_rdap.ap[0][0], 128], [1, ncd], [0, D_IN]],
                )
                nc.gpsimd.tensor_tensor(
                    out=outv_s[:, DEPTH * d:DEPTH * d + ncd, :],
                    in0=arena[:, d, 0:ncd, 0:64],
                    in1=rdb,
                    op=Alu.mult,
                )
                # rd -> DRAM in (group, slot) order; then broadcast-replicate
                for j in range(COLG):
                    rd_src = bass.AP(
                        tensor=_rdap.tensor,
                        offset=_rdap.offset + 64 * j * _rdap.ap[0][0] + DEPTH * d,
                        ap=[[_rdap.ap[0][0], G_TGT], [1, ncd]],
                    )
                    rd_dst = bass.AP(
                        tensor=rdflat_d,
                        offset=GPD * G_TGT * d + G_TGT * j,
                        ap=[[1, G_TGT], [COLG * G_TGT, ncd]],
                    )
                    nc.sync.dma_start(out=rd_dst, in_=rd_src)
                rdx = p2p.tile([128, GPD * G_TGT], bf16)
                nc.sync.dma_start(
                    out=rdx[:, 0:gn * G_TGT],
                    in_=bass.AP(
                        tensor=rdflat_d, offset=GPD * G_TGT * d,
                        ap=[[0, 128], [1, gn * G_TGT]],
                    ),
                )
                _rr = rdx[:]
                rdap2 = bass.AP(
                    tensor=_rr.tensor,
                    offset=_rr.offset,
                    ap=[[_rr.ap[0][0], 128], [G_TGT, gn], [0, G_TILES], [1, G_TGT]],
                )
                scr2 = p2p.tile([128, GPD, G_TILES, G_TGT], bf16)
                nc.vector.tensor_tensor(
                    out=scr2[:, 0:gn, :, :],
                    in0=oh_s[:, g0 * G_TILES:g0 * G_TILES + ntile, :].rearrange(
                        "p (g t) s -> p g t s", g=gn
                    ),
                    in1=rdap2,
                    op=Alu.mult,
                )
                nc.vector.tensor_reduce(
                    out=aw_raw[:, g0 * G_TILES:g0 * G_TILES + ntile].rearrange(
                        "p (g t) -> p g t", g=gn
                    ),
                    in_=scr2[:, 0:gn, :, :],
                    axis=mybir.AxisListType.X,
                    op=Alu.add,
                )

            next_d = 0
            op_arenas = {}
            for w in range(n_w2):
                g0 = W2G * w
                xg_t = loads.tile([128, W2G * G_TILES, 65], bf16)
                nc.sync.dma_start(
                    out=xg_t[:],
                    in_=xg_d[:, g0 * G_TILES:(g0 + W2G) * G_TILES, :],
                )
                at_t = loads.tile([D_EDGE, W2G * E_GROUP], bf16)
                nc.sync.dma_start(
                    out=at_t[:], in_=attr_d[:, g0 * E_GROUP:(g0 + W2G) * E_GROUP]
                )

                # layer 1: h = W1^T @ attr; per half-window (2 groups, 2 banks)
                h_sb = work.tile([D_HID, W2G * E_GROUP], bf16)
                for hw in range(W2G // 2):
                    h_ps = psum_h.tile([D_HID, 2, E_GROUP], f32, space="PSUM")
                    for i in range(2):
                        nc.tensor.matmul(
                            out=h_ps[:, i, :],
                            lhsT=w1_s[:],
                            rhs=at_t[:, (2 * hw + i) * E_GROUP:(2 * hw + i + 1) * E_GROUP],
                            start=True, stop=True,
                        )
                    nc.scalar.activation(
                        out=h_sb[:, 2 * hw * E_GROUP:(2 * hw + 2) * E_GROUP],
                        in_=h_ps[:].rearrange("p a b -> p (a b)"),
                        func=Act.Tanh, bias=b1_s[:], scale=1.0,
                    )

                # layer 2 into b2-preloaded psum: one bank per window
                ew_ps = psum_e.tile([128, W2G * G_TILES, 64], f32, space="PSUM")
                for i in range(W2G // 2):
                    nc.tensor.matmul(
                        out=ew_ps[:, i * 8:(i + 1) * 8, :].rearrange(
                            "p a b -> p (a b)"
                        ),
                        lhsT=ones_s[:],
                        rhs=b2r_s[:],
                        start=True, stop=False,
                        skip_group_check=True,
                    )
                for t in range(W2G * G_TILES):
                    nc.tensor.matmul(
                        out=ew_ps[:, t, :],
                        lhsT=h_sb[:, t * 128:(t + 1) * 128],
                        rhs=w2c_s[:],
                        start=False, stop=(t == W2G * G_TILES - 1),
                        skip_group_check=True,
                    )

                # msg = xg*(ew+b2), written in place over xg (col 64 stays
                # the DMA-loaded ones/denominator column)
                msg_t = xg_t
                nc.vector.tensor_tensor(
                    out=xg_t[:, :, 0:64],
                    in0=xg_t[:, :, 0:64],
                    in1=ew_ps[:],
                    op=Alu.mult,
                )
                msg_tiles[g0] = msg_t
                # score = sum_f msg*attn  (== sum_f xg*(ew+b2)*attn exactly)
                scr_t = work.tile([128, W2G * G_TILES, 64], bf16)
                nc.vector.tensor_tensor(
                    out=scr_t[:],
                    in0=msg_t[:, :, 0:64],
                    in1=bass.AP(
                        tensor=_attnrep.tensor,
                        offset=_attnrep.offset,
                        ap=[[_attnrep.ap[0][0], 128], [0, W2G * G_TILES], [1, 64]],
                    ),
                    op=Alu.mult,
                )
                score_t = work.tile([128, W2G * G_TILES], f32)
                nc.vector.tensor_reduce(
                    out=score_t[:],
                    in_=scr_t[:],
                    axis=mybir.AxisListType.X,
                    op=Alu.add,
                )
                nc.scalar.activation(
                    out=ex_s[:, g0 * G_TILES:(g0 + W2G) * G_TILES],
                    in_=score_t[:],
                    func=Act.Exp,
                )
                # O' = onehot * ex for this window's tiles (off the drain
                # critical chain: computed early, gpsimd, SBUF only)
                dcur = g0 // GPD
                if dcur not in op_arenas:
                    op_arenas[dcur] = opp.tile([128, GPD * G_TILES, G_TGT], bf16, name="opar")
                _ex = ex_s[:]
                t0g = g0 * G_TILES
                lt0 = t0g - dcur * GPD * G_TILES
                exb = bass.AP(
                    tensor=_ex.tensor,
                    offset=_ex.offset + t0g,
                    ap=[[_ex.ap[0][0], 128], [1, W2G * G_TILES], [0, G_TGT]],
                )
                nc.gpsimd.tensor_tensor(
                    out=op_arenas[dcur][:, lt0:lt0 + W2G * G_TILES, :],
                    in0=oh_s[:, t0g:t0g + W2G * G_TILES, :],
                    in1=exb,
                    op=Alu.mult,
                )
                # issue any drain window whose groups are now all available
                while next_d < N_WIN and (
                    min((next_d + 1) * GPD, N_GROUPS) <= (w + 1) * W2G
                ):
                    issue_drain(next_d)
                    next_d += 1
            assert next_d == N_WIN

            # ---- final: output DMAs + attn weight scaling ----
            nc.sync.dma_start(out=outv_d[:], in_=outv_s[:])
            aw = singles.tile([128, N_TILES], f32)
            nc.vector.tensor_tensor(
                out=aw[:], in0=aw_raw[:], in1=ex_s[:], op=Alu.mult
            )
            nc.sync.dma_start(out=attnw_d[:], in_=aw[:])

    nc.finalize()
    return nc


def kernel(x, edge_index, edge_attr, W1, b1, W2, b2, attn_v):
    from concourse.bass_utils import run_bass_kernel_spmd

    x = np.asarray(x, np.float32)
    edge_index = np.asarray(edge_index)
    edge_attr = np.asarray(edge_attr, np.float32)
    W1 = np.asarray(W1, np.float32)
    b1 = np.asarray(b1, np.float32)
    W2 = np.asarray(W2, np.float32)
    b2 = np.asarray(b2, np.float32)
    attn_v = np.asarray(attn_v, np.float32)

    src = np.asarray(edge_index[0], np.int64)
    tgt = np.asarray(edge_index[1], np.int64)
    av = attn_v[:, 0]

    b2row = np.tile(b2, 8)[None, :].astype(BF16)  # [1, 512]

    in_maps = []
    metas = []
    for k in range(N_CORES):
        m, meta = _prep_core(k, x, src, tgt, edge_attr)
        m["W1"] = W1.astype(BF16)
        m["b1"] = b1.reshape(D_HID, 1).astype(np.float32)
        m["W2cat"] = W2.astype(BF16)
        m["b2row"] = b2row
        m["attnv"] = av[None, :].astype(BF16)
        in_maps.append(m)
        metas.append(meta)

    if "nc" not in _CACHED:
        _CACHED["nc"] = _build_graph()
    nc = _CACHED["nc"]

    _CACHED["in_maps"] = in_maps
    res = run_bass_kernel_spmd(nc, in_maps, core_ids=list(range(N_CORES)))
    results = res.results

    out = np.zeros((N_NODES, D_IN), np.float32)
    attnw = np.zeros((N_EDGES,), np.float32)
    for k in range(N_CORES):
        meta = metas[k]
        outv = np.asarray(results[k]["outv"], np.float32)   # [128, 100, 64]
        aw = np.asarray(results[k]["attnw"], np.float32)    # [128, 800]
        out[k * T_PER_CORE:(k + 1) * T_PER_CORE] = outv[
            meta["t_part"], meta["t_col"], :
        ]
        attnw[meta["eids"]] = aw[meta["e_p"], meta["e_T"]]
    return out, attnw


# revision 55
# speedup vs baseline: 1.0173x; 1.0173x over previous
"""CFConv (SchNet-style continuous-filter conv with per-target segment softmax)
on 8 Trainium2 NeuronCores.

Sharding: target-bucketed edge parallelism. Core k owns target nodes
[k*6250, (k+1)*6250). The host buckets+sorts edges by target and bin-packs
targets into 200 groups of <=40 target slots x 512 edge slots (4 tiles of
128), an identical static structure on every core (SPMD-safe, no
collectives: each core owns disjoint targets).

Device pipeline per 2-group window:
  - edge MLP layer 1 on PE (attr_T @ W1), tanh+b1 on ACT
  - layer 2 per 128-edge tile on PE: [ew | ew*attn] = h_tile^T @ [W2 | W2*attn],
    accumulated on top of a PSUM preload of [b2 | b2*attn] (ones outer product)
  - messages msg = xg * (ew + b2) and scores = sum_f xg*(ew*attn + b2*attn)
    as batched DVE ops (xg = host-pregathered x[source])
  - softmax without max-subtraction (scores are O(10); exp safe in f32)
  - segment sums via one-hot matmuls: numer[t,:] | denom[t] accumulate in
    PSUM (one-hot lhsT scaled by exp(score) so pads contribute nothing)
  - attention weights = ex * (O @ (1/denom)) via batched TT+reduce
"""

import math
import sys

import numpy as np

sys.path.insert(0, "/opt/trn_rl_repo")

import ml_dtypes  # noqa: E402

BF16 = ml_dtypes.bfloat16

# problem constants (hardcoded per spec)
N_NODES = 50000
N_EDGES = 800000
D_IN = 64
D_EDGE = 32
D_HID = 128
N_CORES = 8
T_PER_CORE = N_NODES // N_CORES  # 6250

# device layout constants
G_TGT = 40          # target slots per group
G_TILES = 4         # 128-edge tiles per group
E_GROUP = G_TILES * 128  # 512 edge slots per group
N_GROUPS = 200      # groups per core; 200*40 = 8000 target slots
N_TILES = N_GROUPS * G_TILES  # 800
E_CAP = N_GROUPS * E_GROUP    # 102400
COLG = 2            # psum column groups (partition offsets 0 / 64)
DEPTH = 3           # psum drain depth
GPD = COLG * DEPTH  # groups per drain window = 6
N_WIN = math.ceil(N_GROUPS / GPD)  # 34 (last window partial: 2 groups)
NCOL = N_GROUPS // COLG  # 100 columns in drained arena (= g//2)
PSW = 65            # psum width per group: 64 numer + 1 denom

_CACHED = {}


def _pack_core(counts):
    """Greedy edge-limited bin packing of sorted targets into groups."""
    g = np.zeros(T_PER_CORE, np.int32)
    s = np.zeros(T_PER_CORE, np.int32)
    cur_g, cur_s, cur_e = 0, 0, 0
    for t in range(T_PER_CORE):
        c = counts[t]
        if cur_s >= G_TGT or cur_e + c > E_GROUP:
            cur_g += 1
            cur_s, cur_e = 0, 0
        assert cur_g < N_GROUPS, "bin packing overflow; increase N_GROUPS"
        assert c <= E_GROUP, "single target exceeds group capacity"
        g[t] = cur_g
        s[t] = cur_s
        cur_s += 1
        cur_e += c
    return g, s


def _prep_core(k, x, src, tgt, edge_attr):
    """Build one core's input arrays + reassembly metadata."""
    sel = np.nonzero((tgt >= k * T_PER_CORE) & (tgt < (k + 1) * T_PER_CORE))[0]
    tl = tgt[sel] - k * T_PER_CORE
    order = np.argsort(tl, kind="stable")
    eids = sel[order].astype(np.int64)
    tl = tl[order]
    counts = np.bincount(tl, minlength=T_PER_CORE)
    tg, ts = _pack_core(counts)

    eg = tg[tl]                       # per-edge group
    es = ts[tl]                       # per-edge slot in group
    grp_start = np.zeros(N_GROUPS + 1, np.int64)
    np.add.at(grp_start, eg + 1, 1)
    grp_start = np.cumsum(grp_start)
    pos_in_grp = np.arange(len(eids)) - grp_start[eg]

    j = eg * E_GROUP + pos_in_grp
    p = (pos_in_grp % 128).astype(np.int64)          # partition
    T = eg * G_TILES + pos_in_grp // 128             # global tile index

    attr_t = np.zeros((D_EDGE, E_CAP), BF16)
    attr_t[:, j] = edge_attr[eids].T

    xs = x[src[eids]]                                 # [e, 64] host gather
    # [128, N_TILES, 65]: cols 0:64 = x[src], col 64 = 1.0 (denom column)
    xgext = np.zeros((128, N_TILES, 65), BF16)
    xgext[p, T, :64] = xs
    xgext[p, T, 64] = BF16(1.0)

    O = np.zeros((128, N_TILES, G_TGT), BF16)
    O[p, T, es] = BF16(1.0)

    # reassembly: target t -> (partition, col) in outv [128, NCOL, 64]
    t_part = 64 * (tg % COLG) + ts
    t_col = tg // COLG
    return {
        "attr_t": attr_t,
        "xgext": xgext,
        "onehot": O,
    }, {
        "eids": eids,
        "e_p": p,
        "e_T": T,
        "t_part": t_part,
        "t_col": t_col,
    }


def _build_graph():
    """Build the SPMD bass graph (shared across cores)."""
    import contextlib

    from concourse import bacc, bass, mybir, tile

    f32 = mybir.dt.float32
    bf16 = mybir.dt.bfloat16
    Alu = mybir.AluOpType
    Act = mybir.ActivationFunctionType

    nc = bacc.Bacc(None, target_bir_lowering=False)

    attr_d = nc.declare_dram_parameter("attr_t", [D_EDGE, E_CAP], bf16, isOutput=False)
    xg_d = nc.declare_dram_parameter("xgext", [128, N_TILES, 65], bf16, isOutput=False)
    oh_d = nc.declare_dram_parameter("onehot", [128, N_TILES, G_TGT], bf16, isOutput=False)
    w1_d = nc.declare_dram_parameter("W1", [D_EDGE, D_HID], bf16, isOutput=False)
    b1_d = nc.declare_dram_parameter("b1", [D_HID, 1], f32, isOutput=False)
    w2c_d = nc.declare_dram_parameter("W2cat", [D_HID, 64], bf16, isOutput=False)
    # [1, 512]: b2 repeated for the 8 tiles of one window (psum preload row)
    b2r_d = nc.declare_dram_parameter("b2row", [1, 512], bf16, isOutput=False)
    attn_d = nc.declare_dram_parameter("attnv", [1, D_IN], bf16, isOutput=False)

    outv_d = nc.declare_dram_parameter("outv", [128, NCOL, D_IN], bf16, isOutput=True)
    attnw_d = nc.declare_dram_parameter("attnw", [128, N_TILES], f32, isOutput=True)

    rdflat_d = nc.dram_tensor("rdscratch", [1, N_GROUPS * G_TGT], bf16)

    W2G = 2  # groups per pipeline window (psum tiles sized for 2 groups)
    n_w2 = N_GROUPS // W2G  # 100

    with tile.TileContext(nc) as tc:
        with contextlib.ExitStack() as ctx:
            singles = ctx.enter_context(tc.tile_pool(name="singles", bufs=1))
            loads = ctx.enter_context(tc.tile_pool(name="loads", bufs=8))
            work = ctx.enter_context(tc.tile_pool(name="work", bufs=2))
            p2p = ctx.enter_context(tc.tile_pool(name="p2p", bufs=2))
            opp = ctx.enter_context(tc.tile_pool(name="opp", bufs=3))
            psum_h = ctx.enter_context(tc.tile_pool(name="psum_h", bufs=2, space="PSUM"))
            psum_e = ctx.enter_context(tc.tile_pool(name="psum_e", bufs=2, space="PSUM"))
            psum_n = ctx.enter_context(tc.tile_pool(name="psum_n", bufs=2, space="PSUM"))

            # persistent tensors
            w1_s = singles.tile([D_EDGE, D_HID], bf16)
            b1_s = singles.tile([D_HID, 1], f32)
            w2c_s = singles.tile([D_HID, 64], bf16)
            b2r_s = singles.tile([1, 512], bf16)
            ones_s = singles.tile([1, 128], bf16)
            attnrep = singles.tile([128, D_IN], bf16)
            _attnrep = attnrep[:]
            oh_s = singles.tile([128, N_TILES, G_TGT], bf16)     # 6.6 MB
            ex_s = singles.tile([128, N_TILES], f32)
            aw_raw = singles.tile([128, N_TILES], f32)
            arena = singles.tile([128, N_WIN, DEPTH, PSW], f32)  # 3.4 MB
            rd_s = singles.tile([128, N_WIN * DEPTH], bf16)
            outv_s = singles.tile([128, NCOL, D_IN], bf16)       # 1.6 MB

            nc.sync.dma_start(out=w1_s[:], in_=w1_d[:])
            nc.sync.dma_start(out=b1_s[:], in_=b1_d[:])
            nc.sync.dma_start(out=w2c_s[:], in_=w2c_d[:])
            nc.sync.dma_start(out=b2r_s[:], in_=b2r_d[:])
            nc.vector.memset(ones_s[:], 1.0)
            nc.sync.dma_start(
                out=attnrep[:],
                in_=bass.AP(tensor=attn_d, offset=0, ap=[[0, 128], [1, D_IN]]),
            )


            # ---- main pass: per 2-group pipeline window ----
            msg_tiles = [None] * N_GROUPS

            def fetch_oh(d):
                g0 = d * GPD
                gn = min(GPD, N_GROUPS - g0)
                ntile = gn * G_TILES
                nc.sync.dma_start(
                    out=oh_s[:, g0 * G_TILES:g0 * G_TILES + ntile, :],
                    in_=oh_d[:, g0 * G_TILES:g0 * G_TILES + ntile, :],
                )

            fetch_oh(0)

            def issue_drain(d):
                g0 = d * GPD
                gn = min(GPD, N_GROUPS - g0)
                ntile = gn * G_TILES
                if d + 1 < N_WIN:
                    fetch_oh(d + 1)
                op_t = op_arenas[d]
                num_ps = psum_n.tile([128, DEPTH, PSW], f32, space="PSUM")
                # interleave the two column groups so adjacent matmuls sit in
                # different array quadrants and pipeline (~4ns apart)
                for gi2 in range((gn + 1) // 2):
                    for t in range(G_TILES):
                        for j in range(2):
                            gi = 2 * gi2 + j
                            if gi >= gn:
                                continue
                            g = g0 + gi
                            jcg = g % COLG
                            dcol = (g // COLG) % DEPTH
                            ps = num_ps[64 * jcg:64 * jcg + G_TGT, dcol, :]
                            mt = msg_tiles[(g // W2G) * W2G]
                            mi = g % W2G
                            nc.tensor.matmul(
                                out=ps[0:G_TGT, :],
                                lhsT=op_t[:, gi * G_TILES + t, :],
                                rhs=mt[:, mi * G_TILES + t, :],
                                start=(t == 0),
                                stop=(t == G_TILES - 1),
                                tile_position=(0, 64 * jcg),
                                skip_group_check=True,
                            )
                nc.scalar.copy(out=arena[:, d, :, :], in_=num_ps[:])

                # --- incremental post: rd, out divide, pass-2 attn weights ---
                ncd = (gn + COLG - 1) // COLG  # valid out columns this window
                nc.gpsimd.tensor_scalar_max(
                    out=rd_s[:, DEPTH * d:DEPTH * d + DEPTH],
                    in0=arena[:, d, :, 64],
                    scalar1=1e-30,
                )
                with nc.allow_low_precision(reason="bf16 softmax denominators, 2e-2 tolerance"):
                    nc.vector.reciprocal(
                        out=rd_s[:, DEPTH * d:DEPTH * d + DEPTH],
                        in_=rd_s[:, DEPTH * d:DEPTH * d + DEPTH],
                    )
                _rdap = rd_s[:]
                rdb = bass.AP(
                    tensor=_rdap.tensor,
                    offset=_rdap.offset + DEPTH * d,
                    ap=[[_rdap.ap[0][0], 128], [1, ncd], [0, D_IN]],
                )
                nc.gpsimd.tensor_tensor(
                    out=outv_s[:, DEPTH * d:DEPTH * d + ncd, :],
                    in0=arena[:, d, 0:ncd, 0:64],
                    in1=rdb,
                    op=Alu.mult,
                )
                # rd -> DRAM in (group, slot) order; then broadcast-replicate
                for j in range(COLG):
                    rd_src = bass.AP(
                        tensor=_rdap.tensor,
                        offset=_rdap.offset + 64 * j * _rdap.ap[0][0] + DEPTH * d,
                        ap=[[_rdap.ap[0][0], G_TGT], [1, ncd]],
                    )
                    rd_dst = bass.AP(
                        tensor=rdflat_d,
                        offset=GPD * G_TGT * d + G_TGT * j,
                        ap=[[1, G_TGT], [COLG * G_TGT, ncd]],
                    )
                    nc.sync.dma_start(out=rd_dst, in_=rd_src)
                rdx = p2p.tile([128, GPD * G_TGT], bf16)
                nc.sync.dma_start(
                    out=rdx[:, 0:gn * G_TGT],
                    in_=bass.AP(
                        tensor=rdflat_d, offset=GPD * G_TGT * d,
                        ap=[[0, 128], [1, gn * G_TGT]],
                    ),
                )
                _rr = rdx[:]
                rdap2 = bass.AP(
                    tensor=_rr.tensor,
                    offset=_rr.offset,
                    ap=[[_rr.ap[0][0], 128], [G_TGT, gn], [0, G_TILES], [1, G_TGT]],
                )
                scr2 = p2p.tile([128, GPD, G_TILES, G_TGT], bf16)
                nc.vector.tensor_tensor(
                    out=scr2[:, 0:gn, :, :],
                    in0=oh_s[:, g0 * G_TILES:g0 * G_TILES + ntile, :].rearrange(
                        "p (g t) s -> p g t s", g=gn
                    ),
                    in1=rdap2,
                    op=Alu.mult,
                )
                nc.vector.tensor_reduce(
                    out=aw_raw[:, g0 * G_TILES:g0 * G_TILES + ntile].rearrange(
                        "p (g t) -> p g t", g=gn
                    ),
                    in_=scr2[:, 0:gn, :, :],
                    axis=mybir.AxisListType.X,
                    op=Alu.add,
                )

            next_d = 0
            op_arenas = {}
            for w in range(n_w2):
                g0 = W2G * w
                xg_t = loads.tile([128, W2G * G_TILES, 65], bf16)
                nc.sync.dma_start(
                    out=xg_t[:],
                    in_=xg_d[:, g0 * G_TILES:(g0 + W2G) * G_TILES, :],
                )
                at_t = loads.tile([D_EDGE, W2G * E_GROUP], bf16)
                nc.sync.dma_start(
                    out=at_t[:], in_=attr_d[:, g0 * E_GROUP:(g0 + W2G) * E_GROUP]
                )

                # layer 1: h = W1^T @ attr; per half-window (2 groups, 2 banks)
                h_sb = work.tile([D_HID, W2G * E_GROUP], bf16)
                for hw in range(W2G // 2):
                    h_ps = psum_h.tile([D_HID, 2, E_GROUP], f32, space="PSUM")
                    for i in range(2):
                        nc.tensor.matmul(
                            out=h_ps[:, i, :],
                            lhsT=w1_s[:],
                            rhs=at_t[:, (2 * hw + i) * E_GROUP:(2 * hw + i + 1) * E_GROUP],
                            start=True, stop=True,
                        )
                    nc.scalar.activation(
                        out=h_sb[:, 2 * hw * E_GROUP:(2 * hw + 2) * E_GROUP],
                        in_=h_ps[:].rearrange("p a b -> p (a b)"),
                        func=Act.Tanh, bias=b1_s[:], scale=1.0,
                    )

                # layer 2 into b2-preloaded psum: one bank per window
                ew_ps = psum_e.tile([128, W2G * G_TILES, 64], f32, space="PSUM")
                for i in range(W2G // 2):
                    nc.tensor.matmul(
                        out=ew_ps[:, i * 8:(i + 1) * 8, :].rearrange(
                            "p a b -> p (a b)"
                        ),
                        lhsT=ones_s[:],
                        rhs=b2r_s[:],
                        start=True, stop=False,
                        skip_group_check=True,
                    )
                for t in range(W2G * G_TILES):
                    nc.tensor.matmul(
                        out=ew_ps[:, t, :],
                        lhsT=h_sb[:, t * 128:(t + 1) * 128],
                        rhs=w2c_s[:],
                        start=False, stop=(t == W2G * G_TILES - 1),
                        skip_group_check=True,
                    )

                # msg = xg*(ew+b2), written in place over xg (col 64 stays
                # the DMA-loaded ones/denominator column)
                msg_t = xg_t
                nc.vector.tensor_tensor(
                    out=xg_t[:, :, 0:64],
                    in0=xg_t[:, :, 0:64],
                    in1=ew_ps[:],
                    op=Alu.mult,
                )
                msg_tiles[g0] = msg_t
                # score = sum_f msg*attn  (== sum_f xg*(ew+b2)*attn exactly)
                scr_t = work.tile([128, W2G * G_TILES, 64], bf16)
                nc.vector.tensor_tensor(
                    out=scr_t[:],
                    in0=msg_t[:, :, 0:64],
                    in1=bass.AP(
                        tensor=_attnrep.tensor,
                        offset=_attnrep.offset,
                        ap=[[_attnrep.ap[0][0], 128], [0, W2G * G_TILES], [1, 64]],
                    ),
                    op=Alu.mult,
                )
                score_t = work.tile([128, W2G * G_TILES], f32)
                nc.vector.tensor_reduce(
                    out=score_t[:],
                    in_=scr_t[:],
                    axis=mybir.AxisListType.X,
                    op=Alu.add,
                )
                nc.scalar.activation(
                    out=ex_s[:, g0 * G_TILES:(g0 + W2G) * G_TILES],
                    in_=score_t[:],
                    func=Act.Exp,
                )
                # O' = onehot * ex for this window's tiles (off the drain
                # critical chain: computed early, gpsimd, SBUF only)
                dcur = g0 // GPD
                if dcur not in op_arenas:
                    op_arenas[dcur] = opp.tile([128, GPD * G_TILES, G_TGT], bf16, name="opar")
                _ex = ex_s[:]
                t0g = g0 * G_TILES
                lt0 = t0g - dcur * GPD * G_TILES
                exb = bass.AP(
                    tensor=_ex.tensor,
                    offset=_ex.offset + t0g,
                    ap=[[_ex.ap[0][0], 128], [1, W2G * G_TILES], [0, G_TGT]],
                )
                nc.gpsimd.tensor_tensor(
                    out=op_arenas[dcur][:, lt0:lt0 + W2G * G_TILES, :],
                    in0=oh_s[:, t0g:t0g + W2G * G_TILES, :],
                    in1=exb,
                    op=Alu.mult,
                )
                # issue any drain window whose groups are now all available
                while next_d < N_WIN and (
                    min((next_d + 1) * GPD, N_GROUPS) <= (w + 1) * W2G
                ):
                    issue_drain(next_d)
                    next_d += 1
            assert next_d == N_WIN

            # ---- final: output DMAs + attn weight scaling ----
            nc.sync.dma_start(out=outv_d[:], in_=outv_s[:])
            aw = singles.tile([128, N_TILES], f32)
            nc.vector.tensor_tensor(
                out=aw[:], in0=aw_raw[:], in1=ex_s[:], op=Alu.mult
            )
            nc.sync.dma_start(out=attnw_d[:], in_=aw[:])

    nc.finalize()
    return nc


def kernel(x, edge_index, edge_attr, W1, b1, W2, b2, attn_v):
    from concourse.bass_utils import run_bass_kernel_spmd

    x = np.asarray(x, np.float32)
    edge_index = np.asarray(edge_index)
    edge_attr = np.asarray(edge_attr, np.float32)
    W1 = np.asarray(W1, np.float32)
    b1 = np.asarray(b1, np.float32)
    W2 = np.asarray(W2, np.float32)
    b2 = np.asarray(b2, np.float32)
    attn_v = np.asarray(attn_v, np.float32)

    src = np.asarray(edge_index[0], np.int64)
    tgt = np.asarray(edge_index[1], np.int64)
    av = attn_v[:, 0]

    b2row = np.tile(b2, 8)[None, :].astype(BF16)  # [1, 512]

    in_maps = []
    metas = []
    for k in range(N_CORES):
        m, meta = _prep_core(k, x, src, tgt, edge_attr)
        m["W1"] = W1.astype(BF16)
        m["b1"] = b1.reshape(D_HID, 1).astype(np.float32)
        m["W2cat"] = W2.astype(BF16)
        m["b2row"] = b2row
        m["attnv"] = av[None, :].astype(BF16)
        in_maps.append(m)
        metas.append(meta)

    if "nc" not in _CACHED:
        _CACHED["nc"] = _build_graph()
    nc = _CACHED["nc"]

    _CACHED["in_maps"] = in_maps
    res = run_bass_kernel_spmd(nc, in_maps, core_ids=list(range(N_CORES)))
    results = res.results

    out = np.zeros((N_NODES, D_IN), np.float32)
    attnw = np.zeros((N_EDGES,), np.float32)
    for k in range(N_CORES):
        meta = metas[k]
        outv = np.asarray(results[k]["outv"], np.float32)   # [128, 100, 64]
        aw = np.asarray(results[k]["attnw"], np.float32)    # [128, 800]
        out[k * T_PER_CORE:(k + 1) * T_PER_CORE] = outv[
            meta["t_part"], meta["t_col"], :
        ]
        attnw[meta["eids"]] = aw[meta["e_p"], meta["e_T"]]
    return out, attnw
